# revision 1
# baseline (speedup 1.0000x reference)
"""MHA kernel for Trainium2, 8-core tensor-parallel (2 heads per core).

Problem (hardcoded): x [2, 2048, 1024] fp32, Wq/Wk/Wv/Wo [1024, 1024],
bq/bk/bv/bo [1024], H=16 heads, DH=64.  out = MHA(x).

Sharding: heads are split 8 ways (2 heads = 128 proj columns per core).
Each core computes its heads' attention output and a partial output
projection (row-parallel Wo); the host sums the 8 partials and adds the
closed-form bias terms (bv @ Wo + bo).

Per-core device pipeline (all big matmuls bf16 in / fp32 accumulate):
  1. Q^T, K^T [128, 4096] = W.T @ x.T            (contract D, psum N=512)
  2. V token-major [tok, 64] per (b, h, ktile), augmented with a ones
     column -> lhsT [128, 65] slots
  3. S^T tiles [128 k, 512 q] = K Q^T; the two heads' K=64 matmuls sit
     on row-groups 0-1 / 2-3 so the PE packs them concurrently
  4. P^T = exp(S^T / 8) on ScalarE (scores in [-3.6, 3.6], no max pass)
  5. O_raw^T + denom = [V|1].T @ P^T             (psum [65, 512])
  6. r2 = 1/denom (both heads), broadcast via one K=2 fp32 matmul
     (eye2), O_norm^T = O_raw^T * r
  7. out^partial [tok 128, 512] = O_norm^T.T @ Wo  (token-major, fp32)
"""

import numpy as np
import ml_dtypes

D = 1024
T = 4096          # B*S tokens
S = 2048
B = 2
NH = 2            # heads per core
DH = 64
NCORES = 8
SCALE = 0.125     # 1/sqrt(DH)

_CACHE = {}


def _build_nc(reps=1):
    import concourse.bacc as bacc
    import concourse.mybir as mybir
    import concourse.tile as tile
    from concourse.hw_specs import get_activation_tables as _gat

    # Pin Exp and Ln to the one table set that holds both, so the
    # table-load placement pass emits a single ACT_TABLE_LOAD instead of
    # thrashing between exp_and_others and natural_log every combo.
    def _pinned_tables(arch):
        out = {}
        for k, fns in _gat(arch).items():
            if k != "natural_log_exp_and_others":
                fns = {f for f in fns if f.name not in ("Exp", "Ln")}
            out[k] = fns
        return out
    bacc.get_activation_tables = _pinned_tables

    dt = mybir.dt
    f32, bf16 = dt.float32, dt.bfloat16

    nc = bacc.Bacc("TRN2", target_bir_lowering=False, debug=False,
                   num_devices=NCORES)

    xT = nc.dram_tensor("xT", [D, T], bf16, kind="ExternalInput")
    wq_d = nc.dram_tensor("wq", [D, 128], bf16, kind="ExternalInput")
    wk_d = nc.dram_tensor("wk", [D, 128], bf16, kind="ExternalInput")
    wv_d = nc.dram_tensor("wv", [D, 128], bf16, kind="ExternalInput")
    wo_d = nc.dram_tensor("wo", [128, D], bf16, kind="ExternalInput")
    bq_d = nc.dram_tensor("bq", [128, 1], f32, kind="ExternalInput")
    bk_d = nc.dram_tensor("bk", [128, 1], f32, kind="ExternalInput")
    outp = nc.dram_tensor("outp", [T, D], f32, kind="ExternalOutput")

    NKT = S // 128        # 16 key tiles per batch
    NQC = S // 512        # 4 query chunks per batch
    NCK = T // 512        # 8 x^T column chunks
    VSLOT = DH + 1        # 65: V columns + ones column

    with tile.TileContext(nc) as tc:
      for _rep in range(reps):
        with (
            tc.tile_pool(name="persist", bufs=1) as pp,
            tc.tile_pool(name="pt", bufs=2) as ptp,
            tc.tile_pool(name="onorm", bufs=2) as onp,
            tc.tile_pool(name="oraw", bufs=2) as orp,
            tc.tile_pool(name="recip", bufs=2) as rcp,
            tc.tile_pool(name="outsb", bufs=3) as osp,
        ):
            # ---- constants / weights ----
            wq = pp.tile([128, D], bf16, tag="wq")
            wk = pp.tile([128, D], bf16, tag="wk")
            wv = pp.tile([128, D], bf16, tag="wv")
            wo = pp.tile([128, D], bf16, tag="wo")
            for w_sb, w_dr in ((wq, wq_d), (wk, wk_d), (wv, wv_d)):
                nc.sync.dma_start(
                    out=w_sb.rearrange("p (t c) -> p t c", c=128),
                    in_=w_dr.ap().rearrange("(t p) c -> p t c", p=128),
                )
            nc.sync.dma_start(out=wo[:, :], in_=wo_d.ap()[:, :])
            bq = pp.tile([128, 1], f32, tag="bq")
            bk = pp.tile([128, 1], f32, tag="bk")
            nc.sync.dma_start(out=bq[:, :], in_=bq_d.ap()[:, :])
            nc.sync.dma_start(out=bk[:, :], in_=bk_d.ap()[:, :])

            with tc.tile_pool(name="mm_ps", bufs=2, space="PSUM") as mmp:
              # ---- x^T d-tiles, loaded in 512-col chunks so the QKV
              # matmuls can start as soon as chunk 0 of all 8 d-tiles lands
              xt = [pp.tile([128, T], bf16, tag=f"xt{d}", name=f"xt{d}")
                    for d in range(8)]
              for nck in range(NCK):
                  cs = slice(nck * 512, (nck + 1) * 512)
                  for d in range(8):
                      nc.sync.dma_start(
                          out=xt[d][:, cs],
                          in_=xT.ap()[d * 128:(d + 1) * 128, cs])

              # ---- Q^T / K^T projections ----
              qt = pp.tile([128, T], bf16, tag="qt")
              kt = pp.tile([128, T], bf16, tag="kt")
              for nck in range(NCK):
                  cs = slice(nck * 512, (nck + 1) * 512)
                  for proj_sb, w_sb, b_sb in ((qt, wq, bq), (kt, wk, bk)):
                      w3 = w_sb.rearrange("p (t c) -> p t c", c=128)
                      ps = mmp.tile([128, 512], f32, tag="mm")
                      for d in range(8):
                          nc.tensor.matmul(
                              ps[:, :], w3[:, d, :], xt[d][:, cs],
                              start=(d == 0), stop=(d == 7),
                          )
                      nc.vector.tensor_scalar_add(proj_sb[:, cs], ps[:, :],
                                                  b_sb[:, :])

              # ---- V token-major (augmented with ones col) ----
              # per batch: [128 tok, (h, kt) slots of 65]
              vtm = []
              for b in range(B):
                  v_sb = pp.tile([128, NH * NKT * VSLOT], bf16, tag=f"v{b}")
                  v4 = v_sb.rearrange("p (h k c) -> p h k c", h=NH, k=NKT)
                  nc.vector.memset(v4[:, :, :, DH:DH + 1], 1.0)
                  vtm.append(v_sb)
              wv3 = wv.rearrange("p (t c) -> p t c", c=128)
              for b in range(B):
                  v4 = vtm[b].rearrange("p (h k c) -> p h k c", h=NH, k=NKT)
                  for k in range(NKT):
                      tok0 = b * S + k * 128
                      ps = mmp.tile([128, 128], f32, tag="mm")
                      for d in range(8):
                          nc.tensor.matmul(
                              ps[:, :], xt[d][:, tok0:tok0 + 128], wv3[:, d, :],
                              start=(d == 0), stop=(d == 7),
                          )
                      nc.vector.tensor_copy(
                          v4[:, :, k, 0:DH],
                          ps.rearrange("p (h c) -> p h c", h=NH)[:, :, :],
                      )

            with (
                tc.tile_pool(name="st_ps", bufs=3, space="PSUM") as stp,
                tc.tile_pool(name="av_ps", bufs=2, space="PSUM") as avp,
            ):
              # ---- attention per (b, qc), heads interleaved ----
              # The outproj matmuls of combo i are emitted AFTER combo i+1's
              # scores/AV matmuls: PE is in-order, and this gives the
              # reciprocal/broadcast chain a full combo to finish without
              # stalling the PE (stalls > 3.4us re-throttle the PE clock).
              def emit_outproj(q0, onorm):
                  for s4 in range(4):
                      for jc in range(2):
                          op = avp.tile([128, 512], f32, tag="av",
                                        name=f"op{q0}_{s4}_{jc}")
                          nc.tensor.matmul(
                              op[:, :], onorm[:, s4 * 128:(s4 + 1) * 128],
                              wo[:, jc * 512:(jc + 1) * 512],
                              start=True, stop=True,
                          )
                          osb = osp.tile([128, 512], f32, tag="outsb",
                                         name=f"osb{q0}_{s4}_{jc}")
                          nc.vector.tensor_copy(osb[:, :], op[:, :])
                          r0 = q0 + s4 * 128
                          nc.sync.dma_start(
                              out=outp.ap()[r0:r0 + 128,
                                            jc * 512:(jc + 1) * 512],
                              in_=osb[:, :],
                          )

              pending = None
              for b in range(B):
                  v4 = vtm[b].rearrange("p (h k c) -> p h k c", h=NH, k=NKT)
                  for qc in range(NQC):
                      q0 = b * S + qc * 512
                      onorm = onp.tile([128, 512], bf16, tag="onorm",
                                       name=f"onorm{b}_{qc}")
                      pt = ptp.tile([128, NH * NKT * 512], bf16, tag="pt",
                                    name=f"pt{b}_{qc}")
                      pt3 = pt.rearrange("p (h k q) -> p h k q", h=NH, k=NKT)
                      # scores: each st tile holds both heads for one ktile;
                      # consecutive matmuls alternate PE row-groups (0-63 vs
                      # 64-127) so the array runs them concurrently
                      for kti in range(NKT):
                          k0 = b * S + kti * 128
                          st = stp.tile([128, 1024], f32, tag="st")
                          for h in range(NH):
                              hp = h * DH
                              nc.tensor.matmul(
                                  st[:, h * 512:(h + 1) * 512],
                                  kt[hp:hp + DH, k0:k0 + 128],
                                  qt[hp:hp + DH, q0:q0 + 512],
                                  start=True, stop=True,
                              )
                          nc.scalar.activation(
                              pt3[:, :, kti, :], st[:, :],
                              mybir.ActivationFunctionType.Exp,
                              scale=SCALE,
                          )
                      for h in range(NH):
                          hp = h * DH
                          av = avp.tile([128, 512], f32, tag="av")
                          for k in range(NKT):
                              nc.tensor.matmul(
                                  av[0:VSLOT, :], v4[:, h, k, :],
                                  pt3[:, h, k, :],
                                  start=(k == 0), stop=(k == NKT - 1),
                              )
                          oraw = orp.tile([VSLOT, 512], f32, tag="oraw")
                          nc.vector.tensor_copy(oraw[:, :], av[0:VSLOT, :])
                          negl = rcp.tile([1, 512], f32, tag="negl")
                          nc.scalar.activation(
                              negl[:, :], oraw[DH:VSLOT, :],
                              mybir.ActivationFunctionType.Ln)
                          recip = rcp.tile([1, 512], f32, tag="recip")
                          nc.scalar.activation(
                              recip[:, :], negl[:, :],
                              mybir.ActivationFunctionType.Exp, scale=-1.0)
                          rb = orp.tile([DH, 512], f32, tag="rb")
                          nc.gpsimd.partition_broadcast(rb[:, :], recip[:, :])
                          nc.vector.tensor_tensor(
                              onorm[hp:hp + DH, :], oraw[0:DH, :],
                              rb[:, :], op=mybir.AluOpType.mult,
                          )
                      if pending is not None:
                          emit_outproj(*pending)
                      pending = (q0, onorm)
              emit_outproj(*pending)

    nc.compile()
    return nc


def _prep_inputs(x, Wq, bq, Wk, bk, Wv, bv, Wo, bo):
    bf16 = ml_dtypes.bfloat16
    xT = np.ascontiguousarray(
        np.asarray(x, dtype=np.float32).reshape(T, D).T).astype(bf16)
    in_maps = []
    for c in range(NCORES):
        cs = slice(c * 128, (c + 1) * 128)
        in_maps.append({
            "xT": xT,
            "wq": np.ascontiguousarray(Wq[:, cs]).astype(bf16),
            "wk": np.ascontiguousarray(Wk[:, cs]).astype(bf16),
            "wv": np.ascontiguousarray(Wv[:, cs]).astype(bf16),
            "wo": np.ascontiguousarray(Wo[cs, :]).astype(bf16),
            "bq": np.ascontiguousarray(bq[cs]).reshape(128, 1).astype(np.float32),
            "bk": np.ascontiguousarray(bk[cs]).reshape(128, 1).astype(np.float32),
        })
    return in_maps


def kernel(x, Wq, bq, Wk, bk, Wv, bv, Wo, bo, _trace=False, _results=None):
    from concourse.bass_utils import run_bass_kernel_spmd

    x = np.asarray(x); Wq = np.asarray(Wq); Wk = np.asarray(Wk)
    Wv = np.asarray(Wv); Wo = np.asarray(Wo)
    bq = np.asarray(bq); bk = np.asarray(bk); bv = np.asarray(bv)
    bo = np.asarray(bo)

    if "nc" not in _CACHE:
        _CACHE["nc"] = _build_nc()
    nc = _CACHE["nc"]

    in_maps = _prep_inputs(x, Wq, bq, Wk, bk, Wv, bv, Wo, bo)
    res = run_bass_kernel_spmd(
        nc, in_maps, core_ids=list(range(NCORES)), trace=_trace)
    if _results is not None:
        _results.append(res)

    acc = np.zeros((T, D), dtype=np.float32)
    for c in range(NCORES):
        acc += np.asarray(res.results[c]["outp"], dtype=np.float32)
    acc += bv.astype(np.float32) @ Wo.astype(np.float32) + bo.astype(np.float32)
    return acc.reshape(B, S, D)



# revision 4
# speedup vs baseline: 1.2330x; 1.2330x over previous
"""MHA kernel for Trainium2, 8-core tensor-parallel (2 heads per core).

Problem (hardcoded): x [2, 2048, 1024] fp32, Wq/Wk/Wv/Wo [1024, 1024],
bq/bk/bv/bo [1024], H=16 heads, DH=64.  out = MHA(x).

Sharding: heads split 8 ways (2 heads = 128 proj columns per core).
Each core computes its heads' attention and a partial row-parallel
output projection; the host sums the 8 partials and adds the
closed-form bias terms (bv @ Wo + bo).

v2 design (per core, all matmuls bf16 in / fp32 accumulate — fp8 was
measured to cost ~2-4e-2 rel err because attention output is a
near-uniform average of zero-mean V, so quantization noise does not
average out):
  - scores S^T per ktile [128, 2x512] via K=64 matmuls, two heads
    packed on PE row groups 0-63 / 64-127.
  - exp on ScalarE (the kernel bottleneck, ~1 elem/lane/cycle,
    FD=1024 per call), P in bf16.
  - AV with an extra ones-column in V (PSUM row 64) for the softmax
    denominator; reciprocal via DVE reciprocal_approx_fast (ScalarE
    stays exp-only); broadcast via GPSIMD partition_broadcast.
  - QKV work (K/Q chunks, token-major V tiles) is interleaved into the
    attention combo streams: ScalarE starts exp'ing ~10us in and the
    PE always has independent work between dependency stalls.
"""

import numpy as np
import ml_dtypes

D = 1024
T = 4096          # B*S tokens
S = 2048
B = 2
NH = 2            # heads per core
DH = 64
NCORES = 8
NKT = S // 128    # 16 key tiles per batch
NQC = S // 512    # 4 query chunks per batch
SLOT = DH + 1     # 65: V columns + ones column
SCALE = 0.125     # 1/sqrt(DH)

_CACHE = {}


def _build_nc():
    import concourse.bacc as bacc
    import concourse.mybir as mybir
    import concourse.tile as tile

    dt = mybir.dt
    f32, bf16 = dt.float32, dt.bfloat16
    MULT = mybir.AluOpType.mult
    EXP = mybir.ActivationFunctionType.Exp

    nc = bacc.Bacc("TRN2", target_bir_lowering=False, debug=False,
                   num_devices=NCORES)

    xT_d = nc.dram_tensor("xT", [D, T], bf16, kind="ExternalInput")
    wq_d = nc.dram_tensor("wq", [D, 128], bf16, kind="ExternalInput")
    wk_d = nc.dram_tensor("wk", [D, 128], bf16, kind="ExternalInput")
    wv_d = nc.dram_tensor("wv", [D, 128], bf16, kind="ExternalInput")
    wo_d = nc.dram_tensor("wo", [128, D], bf16, kind="ExternalInput")
    bq_d = nc.dram_tensor("bq", [128, 1], f32, kind="ExternalInput")
    bk_d = nc.dram_tensor("bk", [128, 1], f32, kind="ExternalInput")
    outp = nc.dram_tensor("outp", [T, D], f32, kind="ExternalOutput")

    with tile.TileContext(nc) as tc:
        with (
            tc.tile_pool(name="persist", bufs=1) as pp,
            tc.tile_pool(name="pt", bufs=2) as ptp,
            tc.tile_pool(name="onorm", bufs=2) as onp,
            tc.tile_pool(name="oraw", bufs=3) as orp,
            tc.tile_pool(name="dn", bufs=4) as dnp,
            tc.tile_pool(name="rb", bufs=3) as rbp,
            tc.tile_pool(name="outsb", bufs=4) as osp,
        ):
            # ---- persistent SBUF tensors ----
            w8 = {}
            for nm, wd in (("q", wq_d), ("k", wk_d), ("v", wv_d)):
                w = pp.tile([128, 8 * 128], bf16, tag=f"w8{nm}")
                nc.sync.dma_start(
                    out=w.rearrange("p (d m) -> p d m", d=8),
                    in_=wd.ap().rearrange("(d p) m -> p d m", p=128),
                )
                w8[nm] = w.rearrange("p (d m) -> p d m", d=8)
            wo = pp.tile([128, D], bf16, tag="wo")
            nc.sync.dma_start(out=wo[:, :], in_=wo_d.ap()[:, :])
            bq = pp.tile([128, 1], f32, tag="bq")
            bk = pp.tile([128, 1], f32, tag="bk")
            nc.sync.dma_start(out=bq[:, :], in_=bq_d.ap()[:, :])
            nc.sync.dma_start(out=bk[:, :], in_=bk_d.ap()[:, :])

            xt = pp.tile([128, 8 * T], bf16, tag="xt")
            x3 = xt.rearrange("p (d c) -> p d c", d=8)
            for nck in range(T // 512):
                cs = slice(nck * 512, (nck + 1) * 512)
                for d in range(8):
                    nc.sync.dma_start(
                        out=x3[:, d, cs],
                        in_=xT_d.ap()[d * 128:(d + 1) * 128, cs])

            qt = pp.tile([128, T], bf16, tag="qt")
            kt = pp.tile([128, T], bf16, tag="kt")

            v4 = []
            for b in range(B):
                v = pp.tile([128, NH * NKT * SLOT], bf16, tag=f"v4_{b}")
                vr = v.rearrange("p (h k c) -> p h k c", h=NH, k=NKT)
                nc.vector.memset(vr[:, :, :, DH:DH + 1], 1.0)
                v4.append(vr)

            with (
                tc.tile_pool(name="st_ps", bufs=2, space="PSUM") as stp,
                tc.tile_pool(name="av_ps", bufs=2, space="PSUM") as avp,
                tc.tile_pool(name="mm_ps", bufs=2, space="PSUM") as mmp,
            ):
                # ---------- building blocks ----------
                def proj_chunk(proj_sb, w3, b_sb, nck):
                    """One 512-col chunk of Q^T/K^T: 8 K=128 matmuls."""
                    cs = slice(nck * 512, (nck + 1) * 512)
                    ps = mmp.tile([128, 512], f32, tag="mm",
                                  name=f"proj{nck}")
                    for d in range(8):
                        nc.tensor.matmul(
                            ps[:, :], w3[:, d, :], x3[:, d, cs],
                            start=(d == 0), stop=(d == 7),
                        )
                    nc.vector.tensor_scalar_add(proj_sb[:, cs], ps[:, :],
                                                b_sb[:, :])

                def v_tile(b, k):
                    """Token-major V tile [128 tok, 128 vdim] -> v4 bf16."""
                    t0 = b * S + k * 128
                    ps = mmp.tile([128, 512], f32, tag="mm", name=f"v{b}_{k}")
                    for d in range(8):
                        nc.tensor.matmul(
                            ps[:, 0:128], x3[:, d, t0:t0 + 128],
                            w8["v"][:, d, :],
                            start=(d == 0), stop=(d == 7),
                        )
                    nc.vector.tensor_copy(
                        v4[b][:, :, k, 0:DH],
                        ps[:, 0:128].rearrange("p (h c) -> p h c", h=NH),
                    )

                def emit_outproj(q0, onorm):
                    for s4 in range(4):
                        for jc in range(2):
                            op = mmp.tile([128, 512], f32, tag="mm",
                                          name=f"op{q0}_{s4}_{jc}")
                            nc.tensor.matmul(
                                op[:, :], onorm[:, s4 * 128:(s4 + 1) * 128],
                                wo[:, jc * 512:(jc + 1) * 512],
                                start=True, stop=True,
                            )
                            osb = osp.tile([128, 512], f32, tag="outsb",
                                           name=f"osb{q0}_{s4}_{jc}")
                            nc.vector.tensor_copy(osb[:, :], op[:, :])
                            r0 = q0 + s4 * 128
                            nc.sync.dma_start(
                                out=outp.ap()[r0:r0 + 128,
                                              jc * 512:(jc + 1) * 512],
                                in_=osb[:, :],
                            )

                # foreign-work queues injected into combo pair slots.
                def mk_slots():
                    return [[] for _ in range(8)]

                inject = {}
                # combo (0,0): K(b0) c1-3 early, V(b0) tiles before their
                # AV pair; Q(b0) c1-3 before the next combos need them.
                sl = mk_slots()
                sl[0].append(lambda: proj_chunk(kt, w8["k"], bk, 1))
                sl[1].append(lambda: proj_chunk(kt, w8["k"], bk, 2))
                sl[2].append(lambda: proj_chunk(kt, w8["k"], bk, 3))
                for i in range(8):
                    sl[i].append(lambda i=i: v_tile(0, 2 * i))
                    sl[i].append(lambda i=i: v_tile(0, 2 * i + 1))
                sl[3].append(lambda: proj_chunk(qt, w8["q"], bq, 1))
                sl[4].append(lambda: proj_chunk(qt, w8["q"], bq, 2))
                sl[5].append(lambda: proj_chunk(qt, w8["q"], bq, 3))
                inject[(0, 0)] = sl
                # combo (0,1): K(b1) chunks 4-7
                sl = mk_slots()
                for i in range(4):
                    sl[2 * i].append(
                        lambda i=i: proj_chunk(kt, w8["k"], bk, 4 + i))
                inject[(0, 1)] = sl
                # combo (0,2): Q(b1) chunks 4-7, V(b1) t0-3
                sl = mk_slots()
                for i in range(4):
                    sl[2 * i].append(
                        lambda i=i: proj_chunk(qt, w8["q"], bq, 4 + i))
                    sl[2 * i + 1].append(lambda i=i: v_tile(1, i))
                inject[(0, 2)] = sl
                # combo (0,3): V(b1) t4-15
                sl = mk_slots()
                for i in range(12):
                    sl[(i * 8) // 12].append(lambda i=i: v_tile(1, 4 + i))
                inject[(0, 3)] = sl

                # ---------- lead-in ----------
                proj_chunk(kt, w8["k"], bk, 0)
                proj_chunk(qt, w8["q"], bq, 0)

                # ---------- attention combos ----------
                pending = None
                for b in range(B):
                    for qc in range(NQC):
                        q0 = b * S + qc * 512
                        slots = inject.get((b, qc), mk_slots())
                        pt = ptp.tile([128, NH * NKT * 512], bf16, tag="pt",
                                      name=f"pt{b}_{qc}")
                        pt4 = pt.rearrange("p (h k q) -> p h k q",
                                           h=NH, k=NKT)
                        onorm = onp.tile([128, 512], bf16, tag="onorm",
                                         name=f"onorm{b}_{qc}")
                        avh = [avp.tile([128, 512], f32, tag="av",
                                        name=f"av{b}_{qc}_{h}")
                               for h in range(NH)]
                        emitted_op = False
                        for i in range(8):
                            for j in range(2):
                                kti = 2 * i + j
                                k0 = b * S + kti * 128
                                st = stp.tile([128, 1024], f32, tag="st")
                                for h in range(NH):
                                    hp = h * DH
                                    nc.tensor.matmul(
                                        st[:, h * 512:(h + 1) * 512],
                                        kt[hp:hp + DH, k0:k0 + 128],
                                        qt[hp:hp + DH, q0:q0 + 512],
                                        start=True, stop=True,
                                    )
                                nc.scalar.activation(
                                    pt4[:, :, kti, :], st[:, :], EXP,
                                    scale=SCALE,
                                )
                            for fn in slots[i]:
                                fn()
                            for h in range(NH):
                                for j in range(2):
                                    kti = 2 * i + j
                                    nc.tensor.matmul(
                                        avh[h][0:SLOT, :],
                                        v4[b][:, h, kti, 0:SLOT],
                                        pt4[:, h, kti, :],
                                        start=(kti == 0), stop=(kti == 15),
                                    )
                            # previous combo's output projection mid-combo
                            if i == 3 and pending is not None:
                                emit_outproj(*pending)
                                pending = None
                                emitted_op = True

                        # softmax normalization chain
                        oraw = [orp.tile([DH, 512], f32, tag="oraw",
                                         name=f"oraw{b}_{qc}_{h}")
                                for h in range(NH)]
                        for h in range(NH):
                            nc.vector.tensor_copy(oraw[h][:, :],
                                                  avh[h][0:DH, :])
                            denom = dnp.tile([1, 512], f32, tag="dn",
                                             name=f"dn{b}_{qc}_{h}")
                            recip = dnp.tile([1, 512], f32, tag="rc",
                                             name=f"rc{b}_{qc}_{h}")
                            nc.vector.tensor_copy(denom[:, :],
                                                  avh[h][DH:DH + 1, :])
                            nc.vector.reciprocal_approx_fast(
                                out=recip[:, :], in_=denom[:, :])
                            hp = h * DH
                            rb = rbp.tile([DH, 512], f32, tag="rb",
                                          name=f"rb{b}_{qc}_{h}")
                            nc.gpsimd.partition_broadcast(
                                rb[:, :], recip[:, :])
                            nc.vector.tensor_tensor(
                                onorm[hp:hp + DH, :], oraw[h][:, :],
                                rb[:, :], op=MULT,
                            )
                        if pending is not None and not emitted_op:
                            emit_outproj(*pending)
                        pending = (q0, onorm)
                emit_outproj(*pending)

    nc.compile()
    return nc


def _prep_inputs(x, Wq, bq, Wk, bk, Wv, bv, Wo, bo):
    bf16 = ml_dtypes.bfloat16
    xT = np.ascontiguousarray(
        np.asarray(x, dtype=np.float32).reshape(T, D).T).astype(bf16)
    in_maps = []
    for c in range(NCORES):
        cs = slice(c * 128, (c + 1) * 128)
        in_maps.append({
            "xT": xT,
            "wq": np.ascontiguousarray(Wq[:, cs]).astype(bf16),
            "wk": np.ascontiguousarray(Wk[:, cs]).astype(bf16),
            "wv": np.ascontiguousarray(Wv[:, cs]).astype(bf16),
            "wo": np.ascontiguousarray(Wo[cs, :]).astype(bf16),
            "bq": np.ascontiguousarray(bq[cs]).reshape(128, 1).astype(np.float32),
            "bk": np.ascontiguousarray(bk[cs]).reshape(128, 1).astype(np.float32),
        })
    return in_maps


def kernel(x, Wq, bq, Wk, bk, Wv, bv, Wo, bo, _trace=False, _results=None):
    from concourse.bass_utils import run_bass_kernel_spmd

    x = np.asarray(x); Wq = np.asarray(Wq); Wk = np.asarray(Wk)
    Wv = np.asarray(Wv); Wo = np.asarray(Wo)
    bq = np.asarray(bq); bk = np.asarray(bk); bv = np.asarray(bv)
    bo = np.asarray(bo)

    if "nc" not in _CACHE:
        _CACHE["nc"] = _build_nc()
    nc = _CACHE["nc"]

    in_maps = _prep_inputs(x, Wq, bq, Wk, bk, Wv, bv, Wo, bo)
    res = run_bass_kernel_spmd(
        nc, in_maps, core_ids=list(range(NCORES)), trace=_trace)
    if _results is not None:
        _results.append(res)

    acc = np.zeros((T, D), dtype=np.float32)
    for c in range(NCORES):
        acc += np.asarray(res.results[c]["outp"], dtype=np.float32)
    acc += bv.astype(np.float32) @ Wo.astype(np.float32) + bo.astype(np.float32)
    return acc.reshape(B, S, D)
